# revision 2
# baseline (speedup 1.0000x reference)
"""Trainium2 Bass kernel for MixLinear GEMM (LLM.int8-style dynamic-quant GEMM
with outlier correction).

Math (per the reference):
    xf        = x.astype(f32).reshape(M, K)
    scale_row = max|xf|/127                     (per row)
    q_x       = round(xf / scale_row)           (RNE, values in [-127, 127])
    acc       = q_x @ q_weight.T                (int-valued f32 GEMM)
    y         = acc * scale_row * w_scale + bias + xf[:, ind] @ weight_cache.T

Sharding: M (rows of x) is split 8 ways; weights are replicated. Each core:
  - computes row scales (DVE abs-max reduce + reciprocal),
  - quantizes x in transposed layout (DVE mult + ACT magic-number RNE),
  - main GEMM in fp16 (int values <= 127 are exact in fp16; PE accumulates fp32),
  - outlier + bias via a small fp16 GEMM (33 contraction rows: 32 outlier
    columns of x + a ones row whose weight row is the bias),
  - fused dequant epilogue on DVE.

Host-side prep is layout only: slab slicing, transposes, and exact dtype
widenings (int8 -> f16). All arithmetic happens on device.
"""

import os

import numpy as np

M, K, N = 8192, 4096, 4096
NCORES = 8
ML = M // NCORES  # 1024 rows per core
MT = ML // 128    # 8 m-tiles per core
KT = K // 128     # 32 k-tiles
NBW = 512         # n-block width (one PSUM bank)
NB = N // NBW     # 8 n-blocks
NO = 32           # outlier columns
MAGIC = float(1.5 * 2**23)  # fp32 add/sub forces round-to-nearest-even int

_nc_cache = {}
last_results = None  # BassKernelResults of the most recent run (for profiling)


def _build_nc(ind_cols):
    from contextlib import ExitStack

    import concourse.bacc as bacc
    import concourse.tile as tile
    from concourse import mybir

    f16 = mybir.dt.float16
    f32 = mybir.dt.float32
    Alu = mybir.AluOpType
    Act = mybir.ActivationFunctionType

    nc = bacc.Bacc("TRN2", target_bir_lowering=False, debug=False,
                   num_devices=NCORES)
    x_nat = nc.dram_tensor("x_nat", [ML, K], f16, kind="ExternalInput").ap()
    xT = nc.dram_tensor("xT", [K, ML], f16, kind="ExternalInput").ap()
    qwT = nc.dram_tensor("qwT", [K, N], f16, kind="ExternalInput").ap()
    wcb = nc.dram_tensor("wcb", [NO + 1, N], f16, kind="ExternalInput").ap()
    wscale = nc.dram_tensor("wscale", [1, N], f32, kind="ExternalInput").ap()
    y = nc.dram_tensor("y", [ML, N], f16, kind="ExternalOutput").ap()

    with tile.TileContext(nc) as tc, ExitStack() as ctx:
        singles = ctx.enter_context(tc.tile_pool(name="singles", bufs=1))
        xnatp = ctx.enter_context(tc.tile_pool(name="xnat", bufs=2))
        xtp = ctx.enter_context(tc.tile_pool(name="xt", bufs=3))
        tp = ctx.enter_context(tc.tile_pool(name="tq", bufs=2))
        t2p = ctx.enter_context(tc.tile_pool(name="tq2", bufs=2))
        qwp = ctx.enter_context(tc.tile_pool(name="qw", bufs=2))
        wcbp = ctx.enter_context(tc.tile_pool(name="wcbp", bufs=2))
        wscp = ctx.enter_context(tc.tile_pool(name="wsc", bufs=2))
        epip = ctx.enter_context(tc.tile_pool(name="epi", bufs=3))
        yp = ctx.enter_context(tc.tile_pool(name="yp", bufs=3))
        psA = ctx.enter_context(tc.tile_pool(name="psA", bufs=4, space="PSUM"))
        psB = ctx.enter_context(tc.tile_pool(name="psB", bufs=2, space="PSUM"))
        dramp = ctx.enter_context(tc.tile_pool(name="dramp", bufs=1, space="DRAM"))

        qxT = singles.tile([128, KT, ML], f16)
        rowmax = singles.tile([128, MT], f32)
        srow = singles.tile([128, MT], f32)
        inv = singles.tile([128, MT], f32)
        invb = singles.tile([128, ML], f32)
        xout = singles.tile([NO + 1, ML], f16)

        # --- row scales: scale_row[m] = absmax(x[m, :]) / 127 ---
        for mt in range(MT):
            xtile = xnatp.tile([128, K], f16)
            nc.sync.dma_start(out=xtile[:], in_=x_nat[mt * 128:(mt + 1) * 128, :])
            nc.vector.tensor_reduce(
                out=rowmax[:, mt:mt + 1], in_=xtile[:],
                op=Alu.max, axis=mybir.AxisListType.X,
                apply_absolute_value=True,
            )
        nc.vector.tensor_scalar_mul(srow[:], rowmax[:], 1.0 / 127.0)
        nc.vector.reciprocal(inv[:], srow[:])
        # inv is [128 part, MT] with row m = mt*128+p; flatten to a [1, ML]
        # DRAM row (elem m at offset m) then partition-broadcast back to SBUF.
        inv_dram = dramp.tile([1, ML], f32)
        nc.sync.dma_start(
            out=inv_dram[:].rearrange("a (t p) -> (a p) t", p=128),
            in_=inv[:],
        )
        nc.gpsimd.dma_start(out=invb[:], in_=inv_dram[:].to_broadcast((128, ML)))

        # --- outlier rows: xout[o, :] = x[:, ind[o]] (= row ind[o] of xT) ---
        for o, col in enumerate(ind_cols):
            nc.sync.dma_start(out=xout[o:o + 1, :], in_=xT[col:col + 1, :])
        nc.vector.memset(xout[NO:NO + 1, :], 1.0)  # ones row -> bias term

        # --- quantize in transposed layout: qxT = RNE(xT * inv) ---
        for kt in range(KT):
            xt_t = xtp.tile([128, ML], f16)
            nc.sync.dma_start(out=xt_t[:], in_=xT[kt * 128:(kt + 1) * 128, :])
            t_t = tp.tile([128, ML], f32)
            nc.vector.tensor_tensor(out=t_t[:], in0=xt_t[:], in1=invb[:],
                                    op=Alu.mult)
            t2_t = t2p.tile([128, ML], f32)
            nc.scalar.activation(out=t2_t[:], in_=t_t[:], func=Act.Copy,
                                 bias=MAGIC)
            nc.scalar.activation(out=qxT[:, kt, :], in_=t2_t[:], func=Act.Copy,
                                 bias=-MAGIC)

        # --- main loop over output column blocks ---
        for nb in range(NB):
            ns = nb * NBW
            qw_t = qwp.tile([128, KT, NBW], f16)
            for g in range(4):
                src = qwT[g * 1024:(g + 1) * 1024, ns:ns + NBW].rearrange(
                    "(kt p) n -> p kt n", p=128)
                nc.sync.dma_start(out=qw_t[:, g * 8:(g + 1) * 8, :], in_=src)
            wcb_t = wcbp.tile([NO + 1, NBW], f16)
            nc.sync.dma_start(out=wcb_t[:], in_=wcb[:, ns:ns + NBW])
            wsc_t = wscp.tile([128, NBW], f32)
            nc.gpsimd.dma_start(out=wsc_t[:],
                                in_=wscale[:, ns:ns + NBW].to_broadcast((128, NBW)))
            for mt in range(MT):
                ms = mt * 128
                pA = psA.tile([128, NBW], f32)
                for kt in range(KT):
                    nc.tensor.matmul(pA[:], lhsT=qxT[:, kt, ms:ms + 128],
                                     rhs=qw_t[:, kt, :],
                                     start=(kt == 0), stop=(kt == KT - 1))
                pB = psB.tile([128, NBW], f32)
                nc.tensor.matmul(pB[:], lhsT=xout[:, ms:ms + 128], rhs=wcb_t[:],
                                 start=True, stop=True)
                te = epip.tile([128, NBW], f32)
                nc.vector.scalar_tensor_tensor(
                    out=te[:], in0=pA[:], scalar=srow[:, mt:mt + 1], in1=wsc_t[:],
                    op0=Alu.mult, op1=Alu.mult)
                y_t = yp.tile([128, NBW], f16)
                nc.vector.tensor_tensor(out=y_t[:], in0=te[:], in1=pB[:],
                                        op=Alu.add)
                nc.sync.dma_start(out=y[ms:ms + 128, ns:ns + NBW], in_=y_t[:])

    nc.compile()
    return nc


def kernel(x, q_weight, scale_col, weight_cache, ind, **_unused):
    global last_results
    from concourse.bass_utils import run_bass_kernel_spmd

    out_dtype = x.dtype  # float16
    xf = np.asarray(x).reshape(M, K)
    qwT_np = np.ascontiguousarray(np.asarray(q_weight).T).astype(np.float16)
    wcb_np = np.ascontiguousarray(np.concatenate(
        [np.asarray(weight_cache).T.astype(np.float16),
         np.asarray(scale_col)[:, 1].astype(np.float16)[None, :]], axis=0))
    wscale_np = np.ascontiguousarray(
        np.asarray(scale_col)[:, 0].astype(np.float32)[None, :])
    ind_cols = tuple(int(i) for i in np.asarray(ind))

    nc = _nc_cache.get(ind_cols)
    if nc is None:
        nc = _build_nc(ind_cols)
        _nc_cache[ind_cols] = nc

    in_maps = []
    for c in range(NCORES):
        slab = xf[c * ML:(c + 1) * ML]
        in_maps.append({
            "x_nat": np.ascontiguousarray(slab),
            "xT": np.ascontiguousarray(slab.T),
            "qwT": qwT_np,
            "wcb": wcb_np,
            "wscale": wscale_np,
        })

    res = run_bass_kernel_spmd(nc, in_maps, core_ids=list(range(NCORES)))
    last_results = res
    out = np.concatenate([res.results[c]["y"] for c in range(NCORES)], axis=0)
    return out.reshape(np.asarray(x).shape).astype(out_dtype, copy=False)
